# revision 4
# baseline (speedup 1.0000x reference)
"""AssignIndex kernel for Trainium2 (8 NeuronCores).

out = arr, except out[index] = element.

Strategy (per sharding hint): shard arr row-wise across the 8 cores
(8192 rows x 1024 f32 = 32 MiB per core). All cores run the identical
SPMD graph: DMA-copy their shard DRAM->DRAM, except local row
`index % rows_per_core-of-owner` which is written from a per-core
"patch" input. For the owner core patch == element; for all other
cores patch == their own original row at that local offset, so the
write is a data no-op and the single SPMD graph stays correct.
"""

import os
import sys

sys.path.insert(0, "/opt/trn_rl_repo")

import numpy as np

N_CORES = 8

# Populated with the most recent BassKernelResults (exec_time_ns etc.)
LAST_RESULT = None


def _build(rows_per_core: int, D: int, local_row: int):
    import concourse.bass as bass
    import concourse.mybir as mybir

    nc = bass.Bass()
    arr = nc.declare_dram_parameter(
        "arr", [rows_per_core, D], mybir.dt.float32, isOutput=False
    )
    patch = nc.declare_dram_parameter(
        "patch", [1, D], mybir.dt.float32, isOutput=False
    )
    out = nc.declare_dram_parameter(
        "out", [rows_per_core, D], mybir.dt.float32, isOutput=True
    )

    # Segments of rows to copy from arr (patched row excluded), balanced
    # across the three DMA-issuing engines (sync/scalar = HWDGE rings,
    # gpsimd = SWDGE ring) so all three queues run concurrently.
    segments = []
    if local_row > 0:
        segments.append((0, local_row))
    if local_row + 1 < rows_per_core:
        segments.append((local_row + 1, rows_per_core))
    total = sum(e - s for s, e in segments)
    n_queues = 3
    # Cut the segment list at cumulative row counts k*total/n_queues so
    # each queue gets contiguous chunks of ~equal total rows.
    cuts = [round(total * k / n_queues) for k in range(1, n_queues)]
    assignments = [[] for _ in range(n_queues)]
    qi, done = 0, 0
    for s, e in segments:
        pos = s
        while pos < e:
            limit = cuts[qi] if qi < len(cuts) else total
            take = min(e - pos, limit - done)
            if take > 0:
                assignments[qi].append((pos, pos + take))
                pos += take
                done += take
            if qi < len(cuts) and done >= cuts[qi]:
                qi += 1

    with (
        nc.Block() as block,
        nc.semaphore("dma_sem") as dma_sem,
        nc.semaphore("dma_sem2") as dma_sem2,
        nc.semaphore("dma_sem3") as dma_sem3,
    ):

        @block.sync
        def _(sync):
            expected = 0
            for s, e in assignments[0]:
                sync.dma_start(out=out[s:e], in_=arr[s:e]).then_inc(dma_sem, 16)
                expected += 16
            sync.dma_start(
                out=out[local_row : local_row + 1], in_=patch[:]
            ).then_inc(dma_sem, 16)
            expected += 16
            sync.wait_ge(dma_sem, expected)

        @block.scalar
        def _(scalar):
            expected = 0
            for s, e in assignments[1]:
                scalar.dma_start(out=out[s:e], in_=arr[s:e]).then_inc(dma_sem2, 16)
                expected += 16
            if expected:
                scalar.wait_ge(dma_sem2, expected)

        @block.gpsimd
        def _(gpsimd):
            expected = 0
            for s, e in assignments[2]:
                gpsimd.dma_start(out=out[s:e], in_=arr[s:e]).then_inc(dma_sem3, 16)
                expected += 16
            if expected:
                gpsimd.wait_ge(dma_sem3, expected)

    return nc


def kernel(arr, index, element):
    global LAST_RESULT
    from concourse.bass_utils import run_bass_kernel_spmd

    arr = np.ascontiguousarray(np.asarray(arr, dtype=np.float32))
    element = np.ascontiguousarray(np.asarray(element, dtype=np.float32))
    N, D = arr.shape
    idx = int(index)
    rows = N // N_CORES
    owner, local = divmod(idx, rows)

    in_maps = []
    for c in range(N_CORES):
        shard = arr[c * rows : (c + 1) * rows]
        p = element if c == owner else shard[local]
        in_maps.append(
            {"arr": shard, "patch": np.ascontiguousarray(p.reshape(1, D))}
        )

    nc = _build(rows, D, local)
    res = run_bass_kernel_spmd(nc, in_maps, core_ids=list(range(N_CORES)))
    LAST_RESULT = res
    return np.concatenate([res.results[c]["out"] for c in range(N_CORES)], axis=0)


# revision 9
# speedup vs baseline: 1.1902x; 1.1902x over previous
"""AssignIndex kernel for Trainium2 (8 NeuronCores).

out = arr, except out[index] = element.

Strategy (per sharding hint): shard arr row-wise across the 8 cores
(8192 rows x 1024 f32 = 32 MiB per core). All cores run the identical
SPMD graph: DMA-copy their shard DRAM->DRAM, except local row
`index % rows_per_core-of-owner` which is written from a per-core
"patch" input. For the owner core patch == element; for all other
cores patch == their own original row at that local offset, so the
write is a data no-op and the single SPMD graph stays correct.
"""

import os
import sys

sys.path.insert(0, "/opt/trn_rl_repo")

import numpy as np

N_CORES = 8

# Populated with the most recent BassKernelResults (exec_time_ns etc.)
LAST_RESULT = None


def _build(rows_per_core: int, D: int, local_row: int):
    import concourse.bass as bass
    import concourse.mybir as mybir

    nc = bass.Bass()
    arr = nc.declare_dram_parameter(
        "arr", [rows_per_core, D], mybir.dt.float32, isOutput=False
    )
    patch = nc.declare_dram_parameter(
        "patch", [1, D], mybir.dt.float32, isOutput=False
    )
    out = nc.declare_dram_parameter(
        "out", [rows_per_core, D], mybir.dt.float32, isOutput=True
    )

    # Segments of rows to copy from arr (patched row excluded), balanced
    # across the three DMA-issuing engines (sync/scalar = HWDGE rings,
    # gpsimd = SWDGE ring) so all three queues run concurrently.
    segments = []
    if local_row > 0:
        segments.append((0, local_row))
    if local_row + 1 < rows_per_core:
        segments.append((local_row + 1, rows_per_core))
    total = sum(e - s for s, e in segments)
    n_queues = int(os.environ.get("K_NQUEUES", "3"))
    n_chunks = int(os.environ.get("K_CHUNKS", "1"))
    # Cut the segment list at cumulative row counts k*total/n_queues so
    # each queue gets contiguous chunks of ~equal total rows.
    cuts = [round(total * k / n_queues) for k in range(1, n_queues)]
    assignments = [[] for _ in range(max(n_queues, 3))]
    qi, done = 0, 0
    for s, e in segments:
        pos = s
        while pos < e:
            limit = cuts[qi] if qi < len(cuts) else total
            take = min(e - pos, limit - done)
            if take > 0:
                assignments[qi].append((pos, pos + take))
                pos += take
                done += take
            if qi < len(cuts) and done >= cuts[qi]:
                qi += 1
    if n_chunks > 1:
        # Split each queue's chunks further so each engine issues several
        # dma_starts (walrus may spread them over more physical queues).
        for q in range(len(assignments)):
            new_chunks = []
            for s, e in assignments[q]:
                size = e - s
                step = max(1, (size + n_chunks - 1) // n_chunks)
                for p in range(s, e, step):
                    new_chunks.append((p, min(p + step, e)))
            assignments[q] = new_chunks

    no_drain = os.environ.get("K_NO_DRAIN", "0") == "1"
    with (
        nc.Block(no_gpsimd_drain=no_drain) as block,
        nc.semaphore("dma_sem") as dma_sem,
        nc.semaphore("dma_sem2") as dma_sem2,
        nc.semaphore("dma_sem3") as dma_sem3,
    ):

        @block.sync
        def _(sync):
            expected = 0
            for s, e in assignments[0]:
                sync.dma_start(out=out[s:e], in_=arr[s:e]).then_inc(dma_sem, 16)
                expected += 16
            sync.dma_start(
                out=out[local_row : local_row + 1], in_=patch[:]
            ).then_inc(dma_sem, 16)
            expected += 16
            sync.wait_ge(dma_sem, expected)

        @block.scalar
        def _(scalar):
            expected = 0
            for s, e in assignments[1]:
                scalar.dma_start(out=out[s:e], in_=arr[s:e]).then_inc(dma_sem2, 16)
                expected += 16
            if expected:
                scalar.wait_ge(dma_sem2, expected)

        @block.gpsimd
        def _(gpsimd):
            expected = 0
            for s, e in assignments[2]:
                gpsimd.dma_start(out=out[s:e], in_=arr[s:e]).then_inc(dma_sem3, 16)
                expected += 16
            if expected:
                gpsimd.wait_ge(dma_sem3, expected)

    return nc


def kernel(arr, index, element):
    global LAST_RESULT
    from concourse.bass_utils import run_bass_kernel_spmd

    arr = np.ascontiguousarray(np.asarray(arr, dtype=np.float32))
    element = np.ascontiguousarray(np.asarray(element, dtype=np.float32))
    N, D = arr.shape
    idx = int(index)
    rows = N // N_CORES
    owner, local = divmod(idx, rows)

    in_maps = []
    for c in range(N_CORES):
        shard = arr[c * rows : (c + 1) * rows]
        p = element if c == owner else shard[local]
        in_maps.append(
            {"arr": shard, "patch": np.ascontiguousarray(p.reshape(1, D))}
        )

    nc = _build(rows, D, local)
    res = run_bass_kernel_spmd(nc, in_maps, core_ids=list(range(N_CORES)))
    LAST_RESULT = res
    return np.concatenate([res.results[c]["out"] for c in range(N_CORES)], axis=0)
